# revision 23
# baseline (speedup 1.0000x reference)
"""Segment-mean pooling (CSR pointer) on 8 Trainium2 NeuronCores.

Strategy (data-parallel over nodes, per sharding hint):
  - Rows of x [N, 128] are split equally across 8 cores (65536 rows each).
  - Host precomputes, per 128-row tile, a one-hot "piece" matrix mapping each
    row to the (<= 8) distinct segments intersecting that tile, and splits
    x into xhi (fp16, subnormals flushed) + xlo (bf16 of the residual):
    x == xhi + xlo to ~2^-21 relative, same total bytes as fp32, and the
    PE runs half-precision matmuls ~3x faster than fp32 (which lowers to
    multiple weight-load passes and disables fast-weight-load).
  - Each core streams its shard through the PE: per tile,
    piece_sums[feat, piece] = xhi.T @ onehot + xlo.T @ onehot, both fp16/bf16
    matmuls accumulating into the same fp32 PSUM group. The onehot is exact
    in both dtypes (entries are 0/1).
  - Host scatter-adds the tiny per-tile piece sums into the [1024, 128]
    segment sums (the "all-reduce over partials"), then divides by counts.
"""

import os
import numpy as np

P = 128            # rows per tile == SBUF partitions
PIECES = 8         # max distinct segments per tile handled on device
CHUNK_T = 32       # tiles per x DMA (32 * 128 * 128 * 2B = 1 MB per dtype)
TILES_PER_BANK = 64  # 64 tiles * 8 pieces * 4B = 2 KB/partition = 1 PSUM bank
XBUFS = 4          # ring depth per hi/lo pool; 2 DMAs/chunk * 4 = lane rotation
N_CORES = 8

_CACHE = {}
LAST_RESULTS = None  # BassKernelResults of the most recent device run


def _prune_implied_waits(nc):
    """Walrus on this compile path allows at most ONE sync wait per
    engine instruction (it has no wait-splitting pass). The Tile layer
    emits semantically-redundant waits: an x-chunk DMA reusing a buffer
    waits both on PE (WAR vs. the matmuls that read the old contents)
    and on the old chunk DMA's completion sem (WAW + lane recycle) —
    but the matmuls themselves waited on that DMA sem, so the PE wait
    transitively implies it.

    Sound pruning rule (pure semaphore arithmetic, order-independent):
    a wait (S >= v) on instruction D is implied by D's wait (A >= va)
    if some instruction whose cumulative post-update value of sem A is
    <= va carries an explicit wait (S >= v') with v' >= v. Sem updates
    post at instruction completion, so A >= va proves that instruction
    completed, hence (S >= v') held, hence (S >= v).
    """
    GE = "sem-ge-imm"
    all_insts = []
    for f in nc.m.functions:
        for blk in f.blocks:
            all_insts.extend(blk.instructions)

    # sems that ever decrease (barriers) are excluded from the arithmetic
    nonmono = set()
    for i in all_insts:
        si = getattr(i, "sync_info", None)
        if si is None:
            continue
        for u in si.on_update or []:
            if u.update_mode not in ("sem-inc", "sem-add-imm"):
                nonmono.add(u.ant_name)

    # Per-engine dispatch is in-order and a wait gates dispatch, so if an
    # instruction's completion post (S = cum) is observed, every wait on
    # any earlier same-engine instruction must have held — and monotonic
    # sems never regress, so those waits still hold. Record each post as
    # (sem, cum, engine, prefix-length into that engine's wait list).
    cum = {}
    eng_waits = {}  # engine -> [(wait_sem, wait_value), ...] in dispatch order
    records = []    # (post_sem, post_cum, engine, n_waits_visible)
    for i in all_insts:
        si = getattr(i, "sync_info", None)
        if si is None:
            continue
        eng = getattr(i, "engine", None)
        lst = eng_waits.setdefault(eng, [])
        for w in si.on_wait or []:
            if w.wait_mode == GE and w.ant_name not in nonmono:
                lst.append((w.ant_name, w.wait_value))
        for u in si.on_update or []:
            if (
                u.update_mode in ("sem-inc", "sem-add-imm")
                and u.ant_name not in nonmono
            ):
                cum[u.ant_name] = cum.get(u.ant_name, 0) + u.update_value
                records.append((u.ant_name, cum[u.ant_name], eng, len(lst)))

    def implied(anchor_sem, anchor_val, s, v):
        # fixpoint closure: known-posted lower bounds per sem
        if anchor_sem in nonmono:
            return False
        best = {anchor_sem: anchor_val}
        kmax = {}
        changed = True
        while changed:
            changed = False
            for ps_, pv, eng, k in records:
                if k > kmax.get(eng, 0) and best.get(ps_, -1) >= pv:
                    for s2, v2 in eng_waits[eng][kmax.get(eng, 0) : k]:
                        if best.get(s2, -1) < v2:
                            best[s2] = v2
                    kmax[eng] = k
                    changed = True
        return best.get(s, -1) >= v

    leftover = []
    for i in all_insts:
        tname = type(i).__name__
        if tname in ("InstDrain", "InstEventSemaphore"):
            continue  # drains are lowered specially; event sems allow 2
        si = getattr(i, "sync_info", None)
        if si is None or not si.on_wait or len(si.on_wait) <= 1:
            continue
        # dedup identical (sem, value) pairs (WAW and lane-recycle collide)
        uniq = {}
        for w in si.on_wait:
            key = (w.ant_name, w.wait_mode, w.wait_value)
            uniq.setdefault(key, w)
        waits = list(uniq.values())
        if len(waits) > 1:
            anchors = sorted(
                waits, key=lambda w: (not w.ant_name.startswith("PE"), w.ant_name)
            )
            for a in anchors:
                if a.wait_mode != GE:
                    continue
                rest = [w for w in waits if w is not a]
                if all(
                    w.wait_mode == GE
                    and implied(a.ant_name, a.wait_value, w.ant_name, w.wait_value)
                    for w in rest
                ):
                    waits = [a]
                    break
        si.on_wait = waits
        if len(waits) > 1:
            leftover.append((tname, getattr(i, "name", "?"), waits))
    if leftover:
        detail = "; ".join(f"{t} {n}: {len(w)} waits" for t, n, w in leftover[:8])
        raise RuntimeError(f"unprunable multi-wait instructions: {detail}")


def _split_drain_waits(nc):
    """Walrus also rejects >1 wait on InstDrain. A drain's waits are a
    pure AND; instructions on one engine queue execute in order, so an
    N-wait drain == N consecutive single-wait drains on that engine."""
    import copy

    for f in nc.m.functions:
        for blk in f.blocks:
            new = []
            for i in blk.instructions:
                si = getattr(i, "sync_info", None)
                if (
                    type(i).__name__ == "InstDrain"
                    and si is not None
                    and si.on_wait
                    and len(si.on_wait) > 1
                ):
                    waits = list(si.on_wait)
                    for k, w in enumerate(waits):
                        c = copy.deepcopy(i)
                        c.sync_info.on_wait = [w]
                        if k < len(waits) - 1:
                            c.sync_info.on_update = []
                        c.name = f"{i.name}s{k}"
                        new.append(c)
                else:
                    new.append(i)
            blk.instructions[:] = new


def _build_program(T, t_process=None, repeat=1, mode="full"):
    """One Bass program, identical on all cores. T = tiles per core.

    t_process < T processes only a prefix; repeat > 1 re-runs the
    DMA+matmul body (intermediate repeats skip the DVE/output stage, so
    PSUM is just overwritten) to amplify device time for wall-clock
    timing. mode="dma" keeps only the x-chunk DMAs (no matmuls);
    mode="pe" keeps only the matmuls (single x chunk reused) — both are
    timing probes. The graded path uses t_process=T, repeat=1, "full".

    Wait-legality plan (1 wait per instruction, see _prune_implied_waits):
      - each chunk issues 2 HW DMAs (xhi, xlo); each pool's ring of
        XBUFS=4 buffers == one full 8-lane DMAHW rotation, and output
        DMAs go via gpsimd (SWDGE lanes), so a chunk DMA's WAW and
        lane-recycle deps collapse onto the SAME (sem, value) — its
        previous-ring-slot DMA — which the pruning pass removes as
        implied by the PE WAR wait.
      - two guard matmuls absorb the onehot-DMA waits, so real matmuls
        carry only their x-chunk RAW wait (PE covered-clock does the rest).
      - psum/outs pools have >= n_banks buffers: no slot reuse, so the
        DVE copy waits only on PE, the out DMA only on DVE.
    """
    import concourse.tile as tile
    from concourse import bass, mybir

    if t_process is None:
        t_process = T
    assert t_process % TILES_PER_BANK == 0 and TILES_PER_BANK % CHUNK_T == 0
    n_banks = t_process // TILES_PER_BANK
    assert n_banks <= 8

    nc = bass.Bass()
    xhi_dram = nc.declare_dram_parameter(
        "xhi", [T * P, P], mybir.dt.float16, isOutput=False
    )
    xlo_dram = nc.declare_dram_parameter(
        "xlo", [T * P, P], mybir.dt.bfloat16, isOutput=False
    )
    ohhi_dram = nc.declare_dram_parameter(
        "ohhi", [P, T, PIECES], mybir.dt.float16, isOutput=False
    )
    ohlo_dram = nc.declare_dram_parameter(
        "ohlo", [P, T, PIECES], mybir.dt.bfloat16, isOutput=False
    )
    out_dram = nc.declare_dram_parameter(
        "out", [P, T, PIECES], mybir.dt.float32, isOutput=True
    )

    # [T*P, 128] row-major -> [p, t, f] view: partition = row-within-tile
    xhr = xhi_dram.rearrange("(t p) f -> p t f", p=P)
    xlr = xlo_dram.rearrange("(t p) f -> p t f", p=P)

    with tile.TileContext(nc) as tc:
        with (
            tc.tile_pool(name="xhin", bufs=XBUFS) as xhpool,
            tc.tile_pool(name="xlin", bufs=XBUFS) as xlpool,
            tc.tile_pool(name="ohh", bufs=1) as ohhpool,
            tc.tile_pool(name="ohl", bufs=1) as ohlpool,
            tc.tile_pool(name="outs", bufs=8) as opool,
            tc.tile_pool(name="psum", bufs=8, space="PSUM") as psum,
        ):
            ohhi_sb = ohhpool.tile([P, T, PIECES], mybir.dt.float16)
            nc.sync.dma_start(ohhi_sb[:], ohhi_dram[:])
            ohlo_sb = ohlpool.tile([P, T, PIECES], mybir.dt.bfloat16)
            nc.sync.dma_start(ohlo_sb[:], ohlo_dram[:])

            chunks_per_bank = TILES_PER_BANK // CHUNK_T
            xh0 = xl0 = None  # single reused chunk in "pe" mode
            for r in range(repeat):
                for b in range(n_banks):
                    ps = None
                    if mode != "dma":
                        ps = psum.tile(
                            [P, TILES_PER_BANK, PIECES], mybir.dt.float32, name="ps"
                        )
                        if r == 0 and b == 0:
                            # guard matmuls: absorb the two onehot-DMA waits;
                            # their output region is overwritten by the first
                            # real matmul (start=True).
                            for g in (ohhi_sb, ohlo_sb):
                                nc.tensor.matmul(
                                    out=ps[0:PIECES, 0, :],
                                    lhsT=g[:, 0, :],
                                    rhs=g[:, 0, :],
                                    start=True,
                                    stop=True,
                                )
                    for cc in range(chunks_per_bank):
                        t0 = b * TILES_PER_BANK + cc * CHUNK_T
                        if mode == "pe":
                            if xh0 is None:
                                xh0 = xhpool.tile(
                                    [P, CHUNK_T, P], mybir.dt.float16, name="xh"
                                )
                                nc.sync.dma_start(xh0[:], xhr[:, 0:CHUNK_T, :])
                                xl0 = xlpool.tile(
                                    [P, CHUNK_T, P], mybir.dt.bfloat16, name="xl"
                                )
                                nc.sync.dma_start(xl0[:], xlr[:, 0:CHUNK_T, :])
                            xh, xl = xh0, xl0
                        else:
                            xh = xhpool.tile(
                                [P, CHUNK_T, P], mybir.dt.float16, name="xh"
                            )
                            nc.sync.dma_start(xh[:], xhr[:, t0 : t0 + CHUNK_T, :])
                            xl = xlpool.tile(
                                [P, CHUNK_T, P], mybir.dt.bfloat16, name="xl"
                            )
                            nc.sync.dma_start(xl[:], xlr[:, t0 : t0 + CHUNK_T, :])
                        if mode == "dma":
                            continue
                        for j in range(CHUNK_T):
                            t = t0 + j
                            tt = t - b * TILES_PER_BANK
                            nc.tensor.matmul(
                                out=ps[:, tt, :],
                                lhsT=xh[:, j, :],
                                rhs=ohhi_sb[:, t, :],
                                start=True,
                                stop=False,
                            )
                            nc.tensor.matmul(
                                out=ps[:, tt, :],
                                lhsT=xl[:, j, :],
                                rhs=ohlo_sb[:, t, :],
                                start=False,
                                stop=True,
                            )
                    if mode != "dma" and r == repeat - 1:
                        ob = opool.tile(
                            [P, TILES_PER_BANK, PIECES], mybir.dt.float32, name="ob"
                        )
                        nc.vector.tensor_copy(ob[:], ps[:])
                        nc.gpsimd.dma_start(
                            out_dram[
                                :, b * TILES_PER_BANK : (b + 1) * TILES_PER_BANK, :
                            ],
                            ob[:],
                        )
            if mode == "dma":
                # out_dram needs a writer; SWDGE casts fp16 -> fp32 in flight.
                nc.gpsimd.dma_start(out_dram[:], ohhi_sb[:])

    nc.finalize()
    _prune_implied_waits(nc)
    _split_drain_waits(nc)
    return nc


def _host_prep(x: np.ndarray, ptr: np.ndarray):
    """Per-tile piece assignment: onehot matrices + piece->segment map,
    plus the fp16-hi / bf16-lo split of x (exact to ~2^-21 relative;
    hi subnormals are flushed on host so a flush-to-zero PE matches)."""
    import ml_dtypes

    N, D = x.shape
    rows_per_core = N // N_CORES
    T = rows_per_core // P          # tiles per core
    NT = N_CORES * T                # total tiles

    # batch[j] = segment of row j (same formula as the reference)
    batch = np.searchsorted(ptr, np.arange(N, dtype=np.int64), side="right") - 1
    batch_t = batch.reshape(NT, P)

    # dense rank of each row's segment within its tile (batch is sorted)
    newseg = np.zeros((NT, P), dtype=np.int64)
    newseg[:, 1:] = batch_t[:, 1:] != batch_t[:, :-1]
    rank = np.cumsum(newseg, axis=1)          # [NT, P], 0..m-1
    n_pieces = rank[:, -1] + 1
    ok = n_pieces <= PIECES                    # tiles the device handles

    # seg_map[g, k] = global segment id of piece k in tile g (-1 = unused)
    seg_map = np.full((NT, PIECES), -1, dtype=np.int64)
    tflat = np.repeat(np.arange(NT), P)
    okflat = np.repeat(ok, P)
    seg_map[tflat[okflat], rank.ravel()[okflat]] = batch_t.ravel()[okflat]

    # onehot[c, r, t_local, k] = 1 iff row r of tile t has rank k
    onehot = np.zeros((N_CORES, P, T, PIECES), dtype=np.float32)
    c_idx = tflat // T
    tl_idx = tflat % T
    r_idx = np.tile(np.arange(P), NT)
    onehot[c_idx[okflat], r_idx[okflat], tl_idx[okflat], rank.ravel()[okflat]] = 1.0

    xhi = x.astype(np.float16)
    hi32 = xhi.astype(np.float32)
    sub = np.abs(hi32) < 2.0 ** -14          # flush fp16 subnormals
    if sub.any():
        xhi[sub] = 0
        hi32[sub] = 0
    xlo = (x - hi32).astype(ml_dtypes.bfloat16)
    dev = {
        "xhi": xhi,
        "xlo": xlo,
        "ohhi": onehot.astype(np.float16),
        "ohlo": onehot.astype(ml_dtypes.bfloat16),
    }
    return T, batch_t, ok, seg_map, dev


def kernel(x: np.ndarray, pointer: np.ndarray) -> np.ndarray:
    global LAST_RESULTS
    from concourse.bass_utils import run_bass_kernel_spmd

    x = np.ascontiguousarray(np.asarray(x, dtype=np.float32))
    ptr = np.asarray(pointer).astype(np.int64)
    N, D = x.shape
    B = ptr.shape[0] - 1
    assert D == P and N % (N_CORES * P) == 0
    rows_per_core = N // N_CORES

    T, batch_t, ok, seg_map, dev = _host_prep(x, ptr)

    key = (T,)
    if key not in _CACHE:
        _CACHE[key] = _build_program(T)
    nc = _CACHE[key]

    in_maps = [
        {
            "xhi": dev["xhi"][c * rows_per_core : (c + 1) * rows_per_core],
            "xlo": dev["xlo"][c * rows_per_core : (c + 1) * rows_per_core],
            "ohhi": dev["ohhi"][c],
            "ohlo": dev["ohlo"][c],
        }
        for c in range(N_CORES)
    ]
    trace = os.environ.get("POOL_KERNEL_TRACE", "0") == "1"
    res = run_bass_kernel_spmd(nc, in_maps, list(range(N_CORES)), trace=trace)
    LAST_RESULTS = res

    seg_sum = np.zeros((B, D), dtype=np.float64)
    for c in range(N_CORES):
        piece = np.asarray(res.results[c]["out"], dtype=np.float64)  # [P(feat), T, 8]
        vals = piece.transpose(1, 2, 0).reshape(T * PIECES, D)       # [(t,k), feat]
        ids = seg_map[c * T : (c + 1) * T].ravel()
        keep = ids >= 0
        np.add.at(seg_sum, ids[keep], vals[keep])

    # host fallback for (vanishingly rare) tiles with > PIECES segments
    for g in np.nonzero(~ok)[0]:
        rows = slice(g * P, (g + 1) * P)
        np.add.at(seg_sum, batch_t[g], x[rows].astype(np.float64))

    counts = (ptr[1:] - ptr[:-1]).astype(np.float64)
    out = seg_sum / np.maximum(counts, 1.0)[:, None]
    return out.astype(np.float32)
